# revision 24
# baseline (speedup 1.0000x reference)
"""Trainium2 Bass kernel for nn_MinimalController (scatter_memory).

Reference computation (B=2048, S=2048, H=64, M=8, V=128):
    h = embed[seq]; wp = sigmoid(h @ wgate_w + b)
    topk_idx = top_k(wp, 8); memory = h[topk_idx]; ctx = h.mean(1)
    rp = sigmoid([ctx, mem] @ rgate_w + b)
    retrieved = (rp*mem).sum(1)/(rp.sum(1)+1e-8); logits = retrieved @ head_w + head_b
    returns (logits, wp, rp)

Key algebraic reduction: every per-token quantity is a function of the token id
(V=128 values only).
  - wp[b,s]   = sig_lut[seq[b,s]]                       (128-entry LUT)
  - ctx only enters via ctx @ rgate_w[:H]: a scalar per row
              = mean_s ctxlut[seq[b,s]]                 (second LUT)
  - memory rows are embed rows of the top-8 tokens; read_prob and logits are
    tiny per-row computations over rank-derived tables.

Device strategy (pure data parallel, B/8 = 256 rows per core, 2 tiles of 128):
  - LUTs evaluated as an exact 128-leaf binary select (mux) tree over the
    7 token bits: 64 leaf-pair injections on the ACT engine
    (out = bit0*delta + base, per-partition scalar APs from a const tensor so
    the program is input-independent), then 63 in-place copy_predicated merges
    on DVE. Exact fp32: outputs are exactly the host-computed fp32 LUT values.
  - top-8 via nc.vector.max (8 largest, desc) + max_index (first unmatched
    occurrence per value) — bit-identical tie semantics to jax.lax.top_k.
  - token ids at the top-8 positions recovered without any gather:
    local_scatter slot markers (8-m) to the top-8 positions, then
    max8(marker*256 + seq) decodes (slot, token) pairs in slot order.
  - read_prob = sigmoid(ctx_score + rg2dot[token] + rgate_b) via a scattered
    segment-one-hot dotted with a replicated rg2dot table.
  - logits via PE: transpose(scattered rp-weighted one-hot) @ (embed@head_w).
"""

import sys

for _p in ("/opt/trn_rl_repo",):
    if _p not in sys.path:
        sys.path.append(_p)

import numpy as np
from concourse import mybir
from concourse.bacc import Bacc
from concourse.tile import TileContext
from concourse.bass_utils import run_bass_kernel_spmd
from concourse.masks import make_identity

F32 = mybir.dt.float32
I32 = mybir.dt.int32
I16 = mybir.dt.int16
I8 = mybir.dt.int8
U32 = mybir.dt.uint32
Alu = mybir.AluOpType
Act = mybir.ActivationFunctionType
Axis = mybir.AxisListType

B, S, H, M, V = 2048, 2048, 64, 8, 128
N_CORES = 8
P = 128


def _sigmoid(x):
    return (1.0 / (1.0 + np.exp(-x.astype(np.float64)))).astype(np.float32)


def host_tables(embed, wgate_w, wgate_b, rgate_w, rgate_b, head_w, head_b):
    """Tiny (V-sized) parameter-derived tables, fp32."""
    embed = np.asarray(embed, np.float32)
    score = (embed @ np.asarray(wgate_w, np.float32))[:, 0] + np.float32(
        np.asarray(wgate_b, np.float32)[0]
    )
    # match jax fp32 sigmoid closely: compute in fp32
    sig_lut = (1.0 / (1.0 + np.exp(-score.astype(np.float32)))).astype(np.float32)
    rg1 = np.asarray(rgate_w, np.float32)[:H, 0]
    rg2 = np.asarray(rgate_w, np.float32)[H:, 0]
    ctxlut = (embed @ rg1).astype(np.float32)
    rg2dot = (embed @ rg2).astype(np.float32)
    tokenlogits = (embed @ np.asarray(head_w, np.float32)).astype(np.float32)
    return sig_lut, ctxlut, rg2dot, tokenlogits


# ---------------------------------------------------------------------------
# Const tensor layout (one [P, CONST_COLS] f32 upload, replicated per core)
# ---------------------------------------------------------------------------
C_SIG_BASE = 0          # 64 cols: sig_lut[2k]
C_SIG_DELTA = 64        # 64 cols: sig_lut[2k+1]-sig_lut[2k]
C_CTX_BASE = 128        # 64
C_CTX_DELTA = 192       # 64
C_RG2SEG = 256          # 1024: rg2dot tiled 8x
C_TL = 1280             # 128: tokenlogits (v on partitions, j on cols)
C_HEADB = 1408          # 128: head_b replicated on all partitions
C_M128 = 1536           # 8: [0,128,...,896]
C_RGATEB = 1544         # 1: rgate_b
CONST_COLS = 1545

# int16 consts (separate tensor: local_scatter needs 2-byte data)
C16_SLOTV = 0           # 8: [8,7,...,1]
C16_ONES = 8            # 8: ones
CONST16_COLS = 16


def _try_exact_delta(a, b):
    """fp32 d with fp32(a + d) == b, or None (ulp-grid mismatch can make it
    impossible; the device evaluates base + bit*delta in exact fp32)."""
    a = np.float32(a)
    b = np.float32(b)
    d = np.float32(b - a)
    for _ in range(8):
        got = np.float32(a + d)
        if got == b:
            return d
        d = np.nextafter(
            d, np.float32(-np.inf) if got > b else np.float32(np.inf),
            dtype=np.float32,
        )
    return None


def lut_modes(lut):
    """Per leaf-pair: ('A', a, d) bit-affine; ('B', b, d') inv-bit-affine;
    ('S', a, b) exact select fallback. All produce exact fp32 LUT values."""
    modes, bases, deltas = [], np.zeros(64, np.float32), np.zeros(64, np.float32)
    for k in range(64):
        a, b = np.float32(lut[2 * k]), np.float32(lut[2 * k + 1])
        d = _try_exact_delta(a, b)
        if d is not None:
            modes.append("A"); bases[k] = a; deltas[k] = d
            continue
        d = _try_exact_delta(b, a)
        if d is not None:
            modes.append("B"); bases[k] = b; deltas[k] = d
            continue
        modes.append("S"); bases[k] = a; deltas[k] = b
    return "".join(modes), bases, deltas


def build_const_array(sig_lut, ctxlut, rg2dot, tokenlogits, head_b, rgate_b):
    sig_modes, sig_base, sig_delta = lut_modes(sig_lut)
    ctx_modes, ctx_base, ctx_delta = lut_modes(ctxlut)
    c = np.zeros((P, CONST_COLS), np.float32)
    c[:, C_SIG_BASE : C_SIG_BASE + 64] = sig_base
    c[:, C_SIG_DELTA : C_SIG_DELTA + 64] = sig_delta
    c[:, C_CTX_BASE : C_CTX_BASE + 64] = ctx_base
    c[:, C_CTX_DELTA : C_CTX_DELTA + 64] = ctx_delta
    c[:, C_RG2SEG : C_RG2SEG + M * V] = np.tile(rg2dot, M)
    c[:, C_TL : C_TL + V] = tokenlogits  # row p = tokenlogits[p, :]
    c[:, C_HEADB : C_HEADB + V] = np.asarray(head_b, np.float32)
    c[:, C_M128 : C_M128 + M] = np.arange(M, dtype=np.float32) * V
    c[:, C_RGATEB] = np.float32(np.asarray(rgate_b, np.float32)[0])
    c16 = np.zeros((P, CONST16_COLS), np.int16)
    c16[:, C16_SLOTV : C16_SLOTV + M] = np.arange(M, 0, -1, dtype=np.int16)
    c16[:, C16_ONES : C16_ONES + M] = 1
    return c, c16


def _emit_tree(nc, pool, bits, nb0, const, base_col, delta_col, s_len, tag, modes):
    """Exact 128-leaf mux tree. Returns the root tile [P, s_len] f32.

    Leaves k (pairs 2k/2k+1) injected on ACT per the host-chosen mode:
      A: bit0*delta + base   B: (1-bit0)*delta + base   S: base then select(b0)
    Merges: in-place copy_predicated on DVE by bits 1..6. All exact fp32.
    """
    stack = []  # list of (level, tile)
    for k in range(64):
        leaf = pool.tile([P, s_len], F32, tag=tag)
        base_ap = const[:, base_col + k : base_col + k + 1]
        delta_ap = const[:, delta_col + k : delta_col + k + 1]
        mode = modes[k]
        if mode == "A":
            nc.scalar.activation(leaf[:], bits[0][:], Act.Identity,
                                 bias=base_ap, scale=delta_ap)
        elif mode == "B":
            nc.scalar.activation(leaf[:], nb0[:], Act.Identity,
                                 bias=base_ap, scale=delta_ap)
        else:  # S: constant fill of a, then predicated overwrite with b
            nc.scalar.activation(leaf[:], bits[0][:], Act.Identity,
                                 bias=base_ap, scale=0.0)
            nc.vector.copy_predicated(
                leaf[:], bits[0][:], delta_ap.to_broadcast([P, s_len])
            )
        node = (0, leaf)
        while stack and stack[-1][0] == node[0]:
            lvl, left = stack.pop()
            # left becomes the merged node: where bit_{lvl+1} is set take right
            nc.vector.copy_predicated(left[:], bits[lvl + 1][:], node[1][:])
            node = (lvl + 1, left)
        stack.append(node)
    assert len(stack) == 1 and stack[0][0] == 6
    return stack[0][1]


def build_program(b_local=B // N_CORES, s_len=S, num_devices=N_CORES, debug=False,
                  null=False, stage="full", sig_modes="A" * 64, ctx_modes="A" * 64):
    # stage: debugging knob — 'trees' | 'max' | 'scatter' | 'full'
    _ORDER = {"trees": 0, "max": 1, "scatter": 2, "full": 3}
    _lvl = _ORDER[stage]
    assert b_local % P == 0
    n_tiles = b_local // P
    half = s_len // 2
    assert half % 2 == 0 and half * 32 < 2**16

    nc = Bacc("TRN2", target_bir_lowering=False, debug=debug, num_devices=num_devices)
    seq_d = nc.dram_tensor("seq", [b_local, s_len], I32, kind="ExternalInput")
    const_d = nc.dram_tensor("consts", [P, CONST_COLS], F32, kind="ExternalInput")
    const16_d = nc.dram_tensor("consts16", [P, CONST16_COLS], I16, kind="ExternalInput")
    wp_d = nc.dram_tensor("wp", [b_local, s_len], F32, kind="ExternalOutput")
    rp_d = nc.dram_tensor("rp", [b_local, M], F32, kind="ExternalOutput")
    logits_d = nc.dram_tensor("logits", [b_local, V], F32, kind="ExternalOutput")

    if null:
        # identical I/O, minimal body: baseline for dispatch+transfer overhead
        with TileContext(nc) as tc:
            with tc.tile_pool(name="sbuf", bufs=1) as pool:
                t = pool.tile([P, s_len], I32)
                nc.sync.dma_start(out=t[:], in_=seq_d.ap()[:P])
                nc.sync.dma_start(out=wp_d.ap().bitcast(I32)[:P], in_=t[:])
                t2 = pool.tile([P, M], F32)
                nc.vector.memset(t2[:], 0.0)
                nc.sync.dma_start(out=rp_d.ap()[:P], in_=t2[:])
                t3 = pool.tile([P, V], F32)
                nc.vector.memset(t3[:], 0.0)
                nc.sync.dma_start(out=logits_d.ap()[:P], in_=t3[:])
        nc.finalize()
        return nc

    with TileContext(nc) as tc:
        with (
            tc.tile_pool(name="persist", bufs=1) as ppool,
            tc.tile_pool(name="work", bufs=1) as wpool,
            tc.tile_pool(name="tree", bufs=9) as tpool,
            tc.tile_pool(name="small", bufs=2) as spool,
            tc.tile_pool(name="psum", bufs=2, space="PSUM") as psum,
        ):
            const = ppool.tile([P, CONST_COLS], F32)
            nc.sync.dma_start(out=const[:], in_=const_d.ap())
            const16 = ppool.tile([P, CONST16_COLS], I16)
            nc.sync.dma_start(out=const16[:], in_=const16_d.ap())
            ident = ppool.tile([P, P], F32)
            make_identity(nc, ident[:])

            for t in range(n_tiles):
                rows = slice(t * P, (t + 1) * P)
                seq32 = wpool.tile([P, s_len], I32, tag="seq")
                nc.sync.dma_start(out=seq32[:], in_=seq_d.ap()[rows])

                # token bits as int8 0/1 (POOL engine)
                bits = []
                for j in range(7):
                    bj = wpool.tile([P, s_len], I32, tag=f"bit{j}")
                    nc.vector.tensor_scalar(
                        bj[:], seq32[:], j, 1,
                        op0=Alu.logical_shift_right, op1=Alu.bitwise_and,
                    )
                    bits.append(bj)
                nb0 = None
                if "B" in sig_modes or "B" in ctx_modes:
                    nb0 = wpool.tile([P, s_len], I32, tag="nb0")
                    nc.vector.tensor_scalar(
                        nb0[:], bits[0][:], 1, None, op0=Alu.bitwise_xor
                    )

                # --- the two LUT trees ---
                wp_t = _emit_tree(nc, tpool, bits, nb0, const, C_SIG_BASE, C_SIG_DELTA, s_len, "tree", sig_modes)
                ctx_t = _emit_tree(nc, tpool, bits, nb0, const, C_CTX_BASE, C_CTX_DELTA, s_len, "tree", ctx_modes)

                nc.sync.dma_start(out=wp_d.ap()[rows], in_=wp_t[:])

                ctxsum = spool.tile([P, 1], F32, tag="ctxsum")
                nc.vector.tensor_reduce(ctxsum[:], ctx_t[:], axis=Axis.X, op=Alu.add)
                ctxbias = spool.tile([P, 1], F32, tag="ctxbias")
                nc.vector.tensor_scalar(
                    ctxbias[:], ctxsum[:], 1.0 / s_len,
                    const[:, C_RGATEB : C_RGATEB + 1],
                    op0=Alu.mult, op1=Alu.add,
                )

                # --- top-8 ---
                if _lvl < 1:
                    continue
                mx = spool.tile([P, M], F32, tag="mx")
                nc.vector.max(mx[:], wp_t[:])
                mxi = spool.tile([P, M], U32, tag="mxi")
                nc.vector.max_index(mxi[:], mx[:], wp_t[:])

                # positions split into two halves for local_scatter (num_elems<2048)
                ge = spool.tile([P, M], F32, tag="ge")
                nc.vector.tensor_scalar(
                    ge[:], mxi[:], float(half), 2.0 * s_len,
                    op0=Alu.is_ge, op1=Alu.mult,
                )  # 0 or 2*s_len
                idx_lo = spool.tile([P, M], I16, tag="idxlo")
                nc.vector.scalar_tensor_tensor(
                    idx_lo[:], ge[:], -1.0, mxi[:], op0=Alu.mult, op1=Alu.add
                )  # mxi - ge  (>=half -> negative -> ignored)
                idx_hi = spool.tile([P, M], I16, tag="idxhi")
                nc.vector.tensor_scalar(
                    idx_hi[:], mxi[:], float(half), None, op0=Alu.subtract
                )

                if _lvl < 2:
                    continue
                slotmap = wpool.tile([P, s_len], I16, tag="slotmap")
                nc.gpsimd.local_scatter(
                    slotmap[:, :half], const16[:, C16_SLOTV : C16_SLOTV + M], idx_lo[:],
                    channels=P, num_elems=half, num_idxs=M,
                )
                nc.gpsimd.local_scatter(
                    slotmap[:, half:], const16[:, C16_SLOTV : C16_SLOTV + M], idx_hi[:],
                    channels=P, num_elems=half, num_idxs=M,
                )

                key = wpool.tile([P, s_len], F32, tag="key")
                nc.vector.scalar_tensor_tensor(
                    key[:], slotmap[:], 256.0, seq32[:], op0=Alu.mult, op1=Alu.add
                )
                k8 = spool.tile([P, M], F32, tag="k8")
                nc.vector.max(k8[:], key[:])

                # decode: slotv = trunc(k8/256); tok = k8 - 256*slotv
                slotq = spool.tile([P, M], I32, tag="slotq")
                nc.vector.tensor_scalar(
                    slotq[:], k8[:], 1.0 / 256.0, None, op0=Alu.mult
                )
                tokf = spool.tile([P, M], F32, tag="tokf")
                nc.vector.scalar_tensor_tensor(
                    tokf[:], slotq[:], -256.0, k8[:], op0=Alu.mult, op1=Alu.add
                )
                idx16 = spool.tile([P, M], I16, tag="idx16")
                nc.vector.tensor_tensor(
                    idx16[:], tokf[:], const[:, C_M128 : C_M128 + M], op=Alu.add
                )

                # read_prob
                if _lvl < 3:
                    continue
                oh = wpool.tile([P, M * V], I16, tag="oh")
                nc.gpsimd.local_scatter(
                    oh[:], const16[:, C16_ONES : C16_ONES + M], idx16[:],
                    channels=P, num_elems=M * V, num_idxs=M,
                )
                wrg = wpool.tile([P, M * V], F32, tag="wrg")
                nc.vector.tensor_tensor(
                    wrg[:], oh[:], const[:, C_RG2SEG : C_RG2SEG + M * V], op=Alu.mult
                )
                rg2v = spool.tile([P, M], F32, tag="rg2v")
                nc.vector.tensor_reduce(
                    rg2v[:], wrg[:].rearrange("p (m v) -> p m v", v=V),
                    axis=Axis.X, op=Alu.add,
                )
                rp = spool.tile([P, M], F32, tag="rp")
                nc.scalar.activation(
                    rp[:], rg2v[:], Act.Sigmoid, bias=ctxbias[:], scale=1.0
                )
                nc.sync.dma_start(out=rp_d.ap()[rows], in_=rp[:])

                # logits: rp-weighted one-hot (exact f32 via broadcast multiply)
                wsum8 = wpool.tile([P, M * V], F32, tag="wsum8")
                nc.vector.tensor_tensor(
                    wsum8[:].rearrange("p (m v) -> p m v", v=V),
                    oh[:].rearrange("p (m v) -> p m v", v=V),
                    rp[:].to_broadcast([P, M, V]),
                    op=Alu.mult,
                )
                wsum = spool.tile([P, V], F32, tag="wsum")
                nc.vector.tensor_reduce(
                    wsum[:], wsum8[:].rearrange("p (m v) -> p v m", v=V),
                    axis=Axis.X, op=Alu.add,
                )
                denom = spool.tile([P, 1], F32, tag="denom")
                nc.vector.tensor_reduce(denom[:], rp[:], axis=Axis.X, op=Alu.add)
                nc.vector.tensor_scalar(
                    denom[:], denom[:], 1e-8, None, op0=Alu.add
                )
                rcp = spool.tile([P, 1], F32, tag="rcp")
                nc.vector.reciprocal(rcp[:], denom[:])

                wsumT_ps = psum.tile([P, V], F32, tag="wsT")
                nc.tensor.transpose(wsumT_ps[:], wsum[:], ident[:])
                wsumT = spool.tile([P, V], F32, tag="wsumT")
                nc.vector.tensor_copy(wsumT[:], wsumT_ps[:])

                lg_ps = psum.tile([P, V], F32, tag="lg")
                nc.tensor.matmul(
                    lg_ps[:], lhsT=wsumT[:], rhs=const[:, C_TL : C_TL + V],
                    start=True, stop=True,
                )
                lg = spool.tile([P, V], F32, tag="lg_sb")
                nc.vector.tensor_scalar(
                    lg[:], lg_ps[:], rcp[:], None, op0=Alu.mult
                )
                nc.vector.tensor_tensor(
                    lg[:], lg[:], const[:, C_HEADB : C_HEADB + V], op=Alu.add
                )
                nc.sync.dma_start(out=logits_d.ap()[rows], in_=lg[:])

    nc.finalize()
    return nc


_PROGRAM_CACHE = {}


def _get_program(key):
    if key not in _PROGRAM_CACHE:
        b_local, s_len, num_devices, sig_modes, ctx_modes = key
        _PROGRAM_CACHE[key] = build_program(
            b_local, s_len, num_devices, sig_modes=sig_modes, ctx_modes=ctx_modes
        )
    return _PROGRAM_CACHE[key]


def kernel(seq, embed, wgate_w, wgate_b, rgate_w, rgate_b, head_w, head_b):
    seq = np.asarray(seq)
    if seq.dtype != np.int32:
        seq = seq.astype(np.int32)
    b, s_len = seq.shape
    assert b % N_CORES == 0
    b_local = b // N_CORES

    sig_lut, ctxlut, rg2dot, tokenlogits = host_tables(
        embed, wgate_w, wgate_b, rgate_w, rgate_b, head_w, head_b
    )
    consts, consts16 = build_const_array(
        sig_lut, ctxlut, rg2dot, tokenlogits, head_b, rgate_b
    )

    sig_modes, _, _ = lut_modes(sig_lut)
    ctx_modes, _, _ = lut_modes(ctxlut)
    nc = _get_program((b_local, s_len, N_CORES, sig_modes, ctx_modes))
    in_maps = [
        {
            "seq": seq[c * b_local : (c + 1) * b_local],
            "consts": consts,
            "consts16": consts16,
        }
        for c in range(N_CORES)
    ]
    res = run_bass_kernel_spmd(nc, in_maps, list(range(N_CORES)))

    logits = np.concatenate([r["logits"] for r in res.results], axis=0)
    wp = np.concatenate([r["wp"] for r in res.results], axis=0)
    rp = np.concatenate([r["rp"] for r in res.results], axis=0)
    return logits, wp, rp


# revision 28
# speedup vs baseline: 4.5995x; 4.5995x over previous
"""Trainium2 Bass kernel for nn_MinimalController (scatter_memory).

Reference computation (B=2048, S=2048, H=64, M=8, V=128):
    h = embed[seq]; wp = sigmoid(h @ wgate_w + b)
    topk_idx = top_k(wp, 8); memory = h[topk_idx]; ctx = h.mean(1)
    rp = sigmoid([ctx, mem] @ rgate_w + b)
    retrieved = (rp*mem).sum(1)/(rp.sum(1)+1e-8); logits = retrieved @ head_w + head_b
    returns (logits, wp, rp)

Key algebraic reduction: every per-token quantity is a function of the token id
(V=128 values only).
  - wp[b,s]   = sig_lut[seq[b,s]]                       (128-entry LUT)
  - ctx only enters via ctx @ rgate_w[:H]: a scalar per row
              = mean_s ctxlut[seq[b,s]]                 (second LUT)
  - memory rows are embed rows of the top-8 tokens; read_prob and logits are
    tiny per-row computations over token-derived tables.

Device strategy (pure data parallel, B/8 = 256 rows per core). This target has
a large fixed cost per engine instruction, so the kernel minimizes instruction
count: a "superline" layout packs 2 rows per partition ([128, 2, 2048] tiles,
row r = h*128 + p), so each elementwise op covers all 256 rows.
  - LUTs evaluated as an exact 128-leaf binary select (mux) tree over the
    7 token bits: 64 leaf-pair injections on the ACT engine (bit0-affine with
    host-nudged deltas so fp32 arithmetic lands exactly on LUT values, with
    inverted-bit / select fallbacks), then 63 in-place copy_predicated merges
    on DVE per LUT. Outputs are bit-exact fp32 LUT values.
  - top-8 via nc.vector.max (8 largest, desc) + max_index (first unmatched
    occurrence per value) — tie semantics identical to jax.lax.top_k.
  - token ids at the top-8 positions recovered without any gather:
    local_scatter slot markers (8-m) to the top-8 positions, then
    max8(marker*256 + seq) decodes (slot, token) pairs in slot order.
  - read_prob = sigmoid(ctx_score + rg2dot[token] + rgate_b) via a scattered
    segment-one-hot dotted with a replicated rg2dot table.
  - logits via PE: transpose(rp-weighted one-hot) @ (embed@head_w).
"""

import sys

for _p in ("/opt/trn_rl_repo",):
    if _p not in sys.path:
        sys.path.append(_p)

import numpy as np
from concourse import mybir
from concourse.bacc import Bacc
from concourse.tile import TileContext
from concourse.bass_utils import run_bass_kernel_spmd
from concourse.masks import make_identity

F32 = mybir.dt.float32
I32 = mybir.dt.int32
I16 = mybir.dt.int16
I8 = mybir.dt.int8
U32 = mybir.dt.uint32
Alu = mybir.AluOpType
Act = mybir.ActivationFunctionType
Axis = mybir.AxisListType

B, S, H, M, V = 2048, 2048, 64, 8, 128
N_CORES = 8
P = 128
NH = 2                      # row-halves per partition (superline factor)
SL = NH * S                 # superline free length (4096)
Q = 1024                    # local_scatter num_elems limit chunk


def host_tables(embed, wgate_w, wgate_b, rgate_w, rgate_b, head_w, head_b):
    """Tiny (V-sized) parameter-derived tables, fp32."""
    embed = np.asarray(embed, np.float32)
    score = (embed @ np.asarray(wgate_w, np.float32))[:, 0] + np.float32(
        np.asarray(wgate_b, np.float32)[0]
    )
    sig_lut = (1.0 / (1.0 + np.exp(-score.astype(np.float32)))).astype(np.float32)
    rg1 = np.asarray(rgate_w, np.float32)[:H, 0]
    rg2 = np.asarray(rgate_w, np.float32)[H:, 0]
    ctxlut = (embed @ rg1).astype(np.float32)
    rg2dot = (embed @ rg2).astype(np.float32)
    tokenlogits = (embed @ np.asarray(head_w, np.float32)).astype(np.float32)
    return sig_lut, ctxlut, rg2dot, tokenlogits


# ---------------------------------------------------------------------------
# Const tensors (replicated per core): one f32 [P, CONST_COLS] + one int16
# ---------------------------------------------------------------------------
C_SIG_BASE = 0              # 64
C_SIG_DELTA = 64            # 64
C_CTX_BASE = 128            # 64
C_CTX_DELTA = 192           # 64
C_RG2SEG = 256              # 2048: rg2dot tiled 16x (2 halves x 8 slots)
C_TL = 2304                 # 128: tokenlogits (v on partitions, j on cols)
C_HEADB = 2432              # 256: head_b tiled 2x (both halves)
C_M128 = 2688               # 16: [(c % 8) * 128]
C_RGATEB = 2704             # 1
CONST_COLS = 2705

C16_SLOTV = 0               # 8: [8,7,...,1]
C16_ONES = 8                # 8: ones
CONST16_COLS = 16


def _try_exact_delta(a, b):
    """fp32 d with fp32(a + d) == b, or None (the device evaluates
    base + bit*delta in exact fp32; an ulp-grid mismatch can make it
    impossible when ulp(d) > ulp(b))."""
    a = np.float32(a)
    b = np.float32(b)
    d = np.float32(b - a)
    for _ in range(8):
        got = np.float32(a + d)
        if got == b:
            return d
        d = np.nextafter(
            d, np.float32(-np.inf) if got > b else np.float32(np.inf),
            dtype=np.float32,
        )
    return None


def lut_modes(lut):
    """Per leaf-pair: ('A', a, d) bit-affine; ('B', b, d') inv-bit-affine;
    ('S', a, b) exact select fallback. All produce exact fp32 LUT values."""
    modes, bases, deltas = [], np.zeros(64, np.float32), np.zeros(64, np.float32)
    for k in range(64):
        a, b = np.float32(lut[2 * k]), np.float32(lut[2 * k + 1])
        d = _try_exact_delta(a, b)
        if d is not None:
            modes.append("A"); bases[k] = a; deltas[k] = d
            continue
        d = _try_exact_delta(b, a)
        if d is not None:
            modes.append("B"); bases[k] = b; deltas[k] = d
            continue
        modes.append("S"); bases[k] = a; deltas[k] = b
    return "".join(modes), bases, deltas


def build_const_array(sig_lut, ctxlut, rg2dot, tokenlogits, head_b, rgate_b):
    sig_modes, sig_base, sig_delta = lut_modes(sig_lut)
    ctx_modes, ctx_base, ctx_delta = lut_modes(ctxlut)
    c = np.zeros((P, CONST_COLS), np.float32)
    c[:, C_SIG_BASE : C_SIG_BASE + 64] = sig_base
    c[:, C_SIG_DELTA : C_SIG_DELTA + 64] = sig_delta
    c[:, C_CTX_BASE : C_CTX_BASE + 64] = ctx_base
    c[:, C_CTX_DELTA : C_CTX_DELTA + 64] = ctx_delta
    c[:, C_RG2SEG : C_RG2SEG + 2 * M * V] = np.tile(rg2dot, 2 * M)
    c[:, C_TL : C_TL + V] = tokenlogits  # row p = tokenlogits[p, :]
    c[:, C_HEADB : C_HEADB + 2 * V] = np.tile(np.asarray(head_b, np.float32), 2)
    c[:, C_M128 : C_M128 + 2 * M] = (np.arange(2 * M) % M).astype(np.float32) * V
    c[:, C_RGATEB] = np.float32(np.asarray(rgate_b, np.float32)[0])
    c16 = np.zeros((P, CONST16_COLS), np.int16)
    c16[:, C16_SLOTV : C16_SLOTV + M] = np.arange(M, 0, -1, dtype=np.int16)
    c16[:, C16_ONES : C16_ONES + M] = 1
    return c, c16


def _emit_tree(nc, pool, bits, nb0, const, base_col, delta_col, tag, modes):
    """Exact 128-leaf mux tree over [P, SL]. Returns the root tile (f32).

    Leaves k (token pairs 2k/2k+1) injected on ACT per the host-chosen mode:
      A: bit0*delta + base   B: (1-bit0)*delta + base   S: base, select(b0)<-b
    Merges: in-place copy_predicated on DVE by bits 1..6. All exact fp32.
    """
    stack = []  # (level, tile)
    for k in range(64):
        leaf = pool.tile([P, SL], F32, tag=tag)
        base_ap = const[:, base_col + k : base_col + k + 1]
        delta_ap = const[:, delta_col + k : delta_col + k + 1]
        mode = modes[k]
        if mode == "A":
            nc.scalar.activation(leaf[:], bits[0][:], Act.Identity,
                                 bias=base_ap, scale=delta_ap)
        elif mode == "B":
            nc.scalar.activation(leaf[:], nb0[:], Act.Identity,
                                 bias=base_ap, scale=delta_ap)
        else:  # S
            nc.scalar.activation(leaf[:], bits[0][:], Act.Identity,
                                 bias=base_ap, scale=0.0)
            nc.vector.copy_predicated(
                leaf[:], bits[0][:], delta_ap.to_broadcast([P, SL])
            )
        node = (0, leaf)
        while stack and stack[-1][0] == node[0]:
            lvl, left = stack.pop()
            nc.vector.copy_predicated(left[:], bits[lvl + 1][:], node[1][:])
            node = (lvl + 1, left)
        stack.append(node)
    assert len(stack) == 1 and stack[0][0] == 6
    return stack[0][1]


def build_program(b_local=B // N_CORES, s_len=S, num_devices=N_CORES,
                  null=False, sig_modes="A" * 64, ctx_modes="A" * 64):
    assert b_local == NH * P and s_len == S

    nc = Bacc("TRN2", target_bir_lowering=False, debug=False, num_devices=num_devices)
    seq_d = nc.dram_tensor("seq", [b_local, s_len], I32, kind="ExternalInput")
    const_d = nc.dram_tensor("consts", [P, CONST_COLS], F32, kind="ExternalInput")
    const16_d = nc.dram_tensor("consts16", [P, CONST16_COLS], I16, kind="ExternalInput")
    wp_d = nc.dram_tensor("wp", [b_local, s_len], F32, kind="ExternalOutput")
    rp_d = nc.dram_tensor("rp", [b_local, M], F32, kind="ExternalOutput")
    logits_d = nc.dram_tensor("logits", [b_local, V], F32, kind="ExternalOutput")

    # DRAM views with row r = h*P + p mapped to (partition p, half h)
    seq_v = seq_d.ap().rearrange("(h p) s -> p h s", h=NH)
    wp_v = wp_d.ap().rearrange("(h p) s -> p h s", h=NH)
    rp_v = rp_d.ap().rearrange("(h p) m -> p h m", h=NH)
    lg_v = logits_d.ap().rearrange("(h p) v -> p h v", h=NH)

    if null:
        with TileContext(nc) as tc:
            with tc.tile_pool(name="sbuf", bufs=1) as pool:
                t = pool.tile([P, s_len], I32)
                nc.sync.dma_start(out=t[:], in_=seq_d.ap()[:P])
                nc.sync.dma_start(out=wp_d.ap().bitcast(I32)[:P], in_=t[:])
                t2 = pool.tile([P, M], F32)
                nc.vector.memset(t2[:], 0.0)
                nc.sync.dma_start(out=rp_d.ap()[:P], in_=t2[:])
                t3 = pool.tile([P, V], F32)
                nc.vector.memset(t3[:], 0.0)
                nc.sync.dma_start(out=logits_d.ap()[:P], in_=t3[:])
        nc.finalize()
        return nc

    with TileContext(nc) as tc:
        with (
            tc.tile_pool(name="persist", bufs=1) as ppool,
            tc.tile_pool(name="tree", bufs=9) as tpool,
            tc.tile_pool(name="small", bufs=1) as spool,
            tc.tile_pool(name="psum", bufs=2, space="PSUM") as psum,
        ):
            const = ppool.tile([P, CONST_COLS], F32)
            nc.sync.dma_start(out=const[:], in_=const_d.ap())
            const16 = ppool.tile([P, CONST16_COLS], I16)
            nc.sync.dma_start(out=const16[:], in_=const16_d.ap())
            ident = ppool.tile([P, P], F32)
            make_identity(nc, ident[:])

            seq32 = tpool.tile([P, SL], I32, tag="tree")
            nc.sync.dma_start(
                out=seq32[:].rearrange("p (h s) -> p h s", h=NH), in_=seq_v
            )

            # token bit masks as int8 0/1 (ACT affine inputs + predicated-copy
            # masks; int32 shift+and on DVE, int32->int8 casts on POOL)
            bits = [None] * 7
            for j in range(7):
                bj32 = tpool.tile([P, SL], I32, tag="tree")
                nc.vector.tensor_scalar(bj32[:], seq32[:], j, 1,
                                        op0=Alu.logical_shift_right, op1=Alu.bitwise_and)
                bj = ppool.tile([P, SL], I8, tag=f"bit{j}")
                nc.gpsimd.tensor_copy(bj[:], bj32[:])
                bits[j] = bj

            nb0 = None
            if "B" in sig_modes or "B" in ctx_modes:
                nb0 = ppool.tile([P, SL], I8)
                nc.vector.tensor_scalar(nb0[:], bits[0][:], 0, None,
                                        op0=Alu.is_equal)

            # --- the two LUT trees (exact fp32) ---
            wp_t = _emit_tree(nc, tpool, bits, nb0, const,
                              C_SIG_BASE, C_SIG_DELTA, "tree", sig_modes)
            ctx_t = _emit_tree(nc, tpool, bits, nb0, const,
                               C_CTX_BASE, C_CTX_DELTA, "tree", ctx_modes)

            nc.sync.dma_start(
                out=wp_v, in_=wp_t[:].rearrange("p (h s) -> p h s", h=NH)
            )

            ctxsum = spool.tile([P, NH], F32)
            nc.vector.tensor_reduce(
                ctxsum[:], ctx_t[:].rearrange("p (h s) -> p h s", h=NH),
                axis=Axis.X, op=Alu.add,
            )
            ctxbias = spool.tile([P, NH], F32)
            nc.vector.tensor_scalar(
                ctxbias[:], ctxsum[:], 1.0 / s_len,
                const[:, C_RGATEB : C_RGATEB + 1],
                op0=Alu.mult, op1=Alu.add,
            )

            # --- top-8 per row-half ---
            mx = spool.tile([P, NH * M], F32)
            mxi = spool.tile([P, NH * M], U32)
            for h in range(NH):
                sl = slice(h * s_len, (h + 1) * s_len)
                ms = slice(h * M, (h + 1) * M)
                nc.vector.max(mx[:, ms], wp_t[:, sl])
                nc.vector.max_index(mxi[:, ms], mx[:, ms], wp_t[:, sl])

            # quarter-relative indices for local_scatter (num_elems <= 1024):
            # lo = mxi - 2048*(mxi >= 1024)  (valid in [0,1024), else negative)
            # hi = mxi - 1024                (valid in [0,1024), else negative)
            ge = spool.tile([P, NH * M], F32)
            nc.vector.tensor_scalar(ge[:], mxi[:], float(Q), 2.0 * s_len,
                                    op0=Alu.is_ge, op1=Alu.mult)
            idx_lo = spool.tile([P, NH * M], I16)
            nc.vector.scalar_tensor_tensor(idx_lo[:], ge[:], -1.0, mxi[:],
                                           op0=Alu.mult, op1=Alu.add)
            idx_hi = spool.tile([P, NH * M], I16)
            nc.vector.tensor_scalar(idx_hi[:], mxi[:], float(Q), None,
                                    op0=Alu.subtract)

            slotmap = tpool.tile([P, SL], I16, tag="tree")
            for quarter in range(SL // Q):
                h, odd = divmod(quarter, 2)
                src = idx_hi if odd else idx_lo
                nc.gpsimd.local_scatter(
                    slotmap[:, quarter * Q : (quarter + 1) * Q],
                    const16[:, C16_SLOTV : C16_SLOTV + M],
                    src[:, h * M : (h + 1) * M],
                    channels=P, num_elems=Q, num_idxs=M,
                )

            seq32b = tpool.tile([P, SL], I32, tag="tree")
            nc.sync.dma_start(
                out=seq32b[:].rearrange("p (h s) -> p h s", h=NH), in_=seq_v
            )
            key = tpool.tile([P, SL], F32, tag="tree")
            nc.vector.scalar_tensor_tensor(key[:], slotmap[:], 256.0, seq32b[:],
                                           op0=Alu.mult, op1=Alu.add)
            k8 = spool.tile([P, NH * M], F32)
            for h in range(NH):
                nc.vector.max(k8[:, h * M : (h + 1) * M],
                              key[:, h * s_len : (h + 1) * s_len])

            # decode (slot, token): slotv = trunc(k8/256); tok = k8 - 256*slotv
            slotq = spool.tile([P, NH * M], I32)
            nc.vector.tensor_scalar(slotq[:], k8[:], 1.0 / 256.0, None, op0=Alu.mult)
            tokf = spool.tile([P, NH * M], F32)
            nc.vector.scalar_tensor_tensor(tokf[:], slotq[:], -256.0, k8[:],
                                           op0=Alu.mult, op1=Alu.add)
            idx16 = spool.tile([P, NH * M], I16)
            nc.vector.tensor_tensor(idx16[:], tokf[:],
                                    const[:, C_M128 : C_M128 + NH * M], op=Alu.add)

            # segment one-hot of the top-8 tokens (per half, 8 segments of V)
            oh = tpool.tile([P, NH * M * V], I16, tag="tree")
            for h in range(NH):
                nc.gpsimd.local_scatter(
                    oh[:, h * M * V : (h + 1) * M * V],
                    const16[:, C16_ONES : C16_ONES + M],
                    idx16[:, h * M : (h + 1) * M],
                    channels=P, num_elems=M * V, num_idxs=M,
                )

            # read_prob = sigmoid(rg2dot[token] + ctx_score + rgate_b)
            wrg = tpool.tile([P, NH * M * V], F32, tag="tree")
            nc.vector.tensor_tensor(
                wrg[:], oh[:], const[:, C_RG2SEG : C_RG2SEG + NH * M * V],
                op=Alu.mult,
            )
            rg2v = spool.tile([P, NH * M], F32)
            nc.vector.tensor_reduce(
                rg2v[:], wrg[:].rearrange("p (c v) -> p c v", v=V),
                axis=Axis.X, op=Alu.add,
            )
            rp = spool.tile([P, NH * M], F32)
            for h in range(NH):
                ms = slice(h * M, (h + 1) * M)
                nc.scalar.activation(rp[:, ms], rg2v[:, ms], Act.Sigmoid,
                                     bias=ctxbias[:, h : h + 1], scale=1.0)
            nc.sync.dma_start(
                out=rp_v, in_=rp[:].rearrange("p (h m) -> p h m", h=NH)
            )

            # logits = (sum_m rp*tokenlogits[tok]) / (sum rp + 1e-8) + head_b
            wsum8 = tpool.tile([P, NH * M * V], F32, tag="tree")
            nc.vector.tensor_tensor(
                wsum8[:].rearrange("p (c v) -> p c v", v=V),
                oh[:].rearrange("p (c v) -> p c v", v=V),
                rp[:].to_broadcast([P, NH * M, V]),
                op=Alu.mult,
            )
            wsum = spool.tile([P, NH * V], F32)
            nc.vector.tensor_reduce(
                wsum[:].rearrange("p (h v) -> p h v", h=NH),
                wsum8[:].rearrange("p (h m v) -> p h v m", h=NH, v=V),
                axis=Axis.X, op=Alu.add,
            )
            denom = spool.tile([P, NH], F32)
            nc.vector.tensor_reduce(
                denom[:], rp[:].rearrange("p (h m) -> p h m", h=NH),
                axis=Axis.X, op=Alu.add,
            )
            nc.vector.tensor_scalar(denom[:], denom[:], 1e-8, None, op0=Alu.add)
            rcp = spool.tile([P, NH], F32)
            nc.vector.reciprocal(rcp[:], denom[:])

            lg = spool.tile([P, NH * V], F32)
            for h in range(NH):
                vs = slice(h * V, (h + 1) * V)
                wsT_ps = psum.tile([P, V], F32, tag="wsT")
                nc.tensor.transpose(wsT_ps[:], wsum[:, vs], ident[:])
                wsT = spool.tile([P, V], F32, tag=f"wsT{h}")
                nc.vector.tensor_copy(wsT[:], wsT_ps[:])
                lg_ps = psum.tile([P, V], F32, tag="lg")
                nc.tensor.matmul(lg_ps[:], lhsT=wsT[:],
                                 rhs=const[:, C_TL : C_TL + V],
                                 start=True, stop=True)
                nc.vector.tensor_scalar(lg[:, vs], lg_ps[:],
                                        rcp[:, h : h + 1], None, op0=Alu.mult)
            nc.vector.tensor_tensor(
                lg[:], lg[:], const[:, C_HEADB : C_HEADB + NH * V], op=Alu.add
            )
            nc.sync.dma_start(
                out=lg_v, in_=lg[:].rearrange("p (h v) -> p h v", h=NH)
            )

    nc.finalize()
    return nc


_PROGRAM_CACHE = {}


def _get_program(key):
    if key not in _PROGRAM_CACHE:
        b_local, s_len, num_devices, sig_modes, ctx_modes = key
        _PROGRAM_CACHE[key] = build_program(
            b_local, s_len, num_devices, sig_modes=sig_modes, ctx_modes=ctx_modes
        )
    return _PROGRAM_CACHE[key]


def kernel(seq, embed, wgate_w, wgate_b, rgate_w, rgate_b, head_w, head_b):
    seq = np.asarray(seq)
    if seq.dtype != np.int32:
        seq = seq.astype(np.int32)
    b, s_len = seq.shape
    assert b % N_CORES == 0
    b_local = b // N_CORES

    sig_lut, ctxlut, rg2dot, tokenlogits = host_tables(
        embed, wgate_w, wgate_b, rgate_w, rgate_b, head_w, head_b
    )
    consts, consts16 = build_const_array(
        sig_lut, ctxlut, rg2dot, tokenlogits, head_b, rgate_b
    )

    sig_modes, _, _ = lut_modes(sig_lut)
    ctx_modes, _, _ = lut_modes(ctxlut)
    nc = _get_program((b_local, s_len, N_CORES, sig_modes, ctx_modes))
    in_maps = [
        {
            "seq": seq[c * b_local : (c + 1) * b_local],
            "consts": consts,
            "consts16": consts16,
        }
        for c in range(N_CORES)
    ]
    res = run_bass_kernel_spmd(nc, in_maps, list(range(N_CORES)))

    logits = np.concatenate([r["logits"] for r in res.results], axis=0)
    wp = np.concatenate([r["wp"] for r in res.results], axis=0)
    rp = np.concatenate([r["rp"] for r in res.results], axis=0)
    return logits, wp, rp
